# revision 26
# baseline (speedup 1.0000x reference)
"""EvidentialUncertaintyDistance Trainium2 kernel (prototype-sharded).

Reference computation (Nq=2048, Np=256, D=128, H1=128, H2=64):
    qh = q @ W1[:, :D].T            [Nq, H1]
    ph = p @ W1[:, D:].T            [Np, H1]
    h1 = relu(qh[:,None,:] + ph[None,:,:] + b1)         [Nq, Np, H1]
    h2 = relu(einsum('qph,oh->qpo', h1, W2) + b2)       [Nq, Np, H2]
    ev = softplus(einsum('qpo,o->qp', h2, W3[0]) + b3)  [Nq, Np]
    base = max(|q|^2 + |p|^2 - 2 q@p.T, 0)
    out  = base / (1/(ev+1) + 1e-8)  ~= base * (ev + 1)

Sharding: PROTOTYPES split across 8 cores (32 p/core, all 2048 q per core);
queries + weights replicated; output assembled along columns. This makes the
per-op free dim 2048 (vs 256 for query sharding), amortizing fixed per-op
engine overheads ~8x on the bias-broadcast stage.

Per-core pipeline:
  - qhT[h,q] (all 2048 q) / phb[h,p_local] via float32r matmuls.
  - stage A (DVE): per local prototype p, h1T[h, 2048q] = relu(qhT +
    (phT+b1)[:,p]) as one bf16 4x-mode tensor_scalar op.
  - mm1 (PE): stationary W2T [128h, 64o] bf16; even p -> psum partitions
    0:64 (array cols 0:64), odd p -> 64:128 via tile_position=(0,64) --
    the two column-group matmuls stream concurrently. psum tile [128, 1024]
    (2 banks) holds h2-pre of a (p-even, p-odd) pair for a 1024-q block,
    stacked (o x p-parity) on partitions.
  - stage C (ACT): h2 = relu(psum + b2stack) -> SBUF bf16, [128, 1024] ops.
  - mm2 (PE): lhsT = h2 slice [K=128=(o x 2p), M=128 q-chunk], rhs =
    block-diag W3 stack [128, 2] -> ev lands directly as [128 q, 2 p]
    columns of one persistent psum bank: ev_ps[:, 32*qchunk + p_local].
  - base (PE, float32r): -2*q@pT_slice (K=128) + rank-2 update (qn x 1 +
    1 x pn) -> base_ps[:, 32*qchunk + p_local].
  - stage D: conf = softplus(ev+b3)+1 via Exp then Ln(x+1) (one ACT table
    set), out = relu(base) * conf -> single strided DMA to out[2048, 32].
"""
import numpy as np
import ml_dtypes
from contextlib import ExitStack

import concourse.bass as bass
import concourse.mybir as mybir
import concourse.tile as tile
from concourse import bacc
from concourse.bass_utils import run_bass_kernel_spmd

F32 = mybir.dt.float32
F32R = mybir.dt.float32r
BF16 = mybir.dt.bfloat16
AF = mybir.ActivationFunctionType
OP = mybir.AluOpType

NQ, NP, D, H1, H2 = 2048, 256, 128, 128, 64
NCORES = 8
NPC = NP // NCORES          # 32 prototypes per core
NPAIR = NPC // 2            # 16 local prototype pairs
NQCHUNK = NQ // 128         # 16 query chunks of 128
MM2_BATCH = 1               # flush mm2 backlog every this many pairs
SKIP_MM2 = False            # timing-ablation knob

_CACHE = {}

# Restrict activation-table choice to natural_log_exp_and_others (contains
# Relu, Exp, Ln, Copy -- everything this kernel uses) so the whole kernel
# needs exactly ONE table load instead of thrashing between the Exp and Ln
# anchor sets at every stage-D alternation. Indices must be preserved
# (act_func_set_id = position), so other sets are emptied, not removed.
from concourse import hw_specs as _hw_specs
if not getattr(_hw_specs, "_act_tables_patched", False):
    _orig_get_act_tables = _hw_specs.get_activation_tables

    def _patched_get_act_tables(module_arch):
        tabs = _orig_get_act_tables(module_arch)
        keep = "natural_log_exp_and_others"
        if keep in tabs:
            tabs = {k: (v if k == keep else set()) for k, v in tabs.items()}
        return tabs

    _hw_specs.get_activation_tables = _patched_get_act_tables
    _hw_specs._act_tables_patched = True
    import concourse.bacc as _bacc_mod
    _bacc_mod.get_activation_tables = _patched_get_act_tables


def build_bass(nrep=1):
    """Build the bass program. nrep>1 traces the whole pipeline nrep times
    into one NEFF (steady-state timing: per-rep marginal cost)."""
    nc = bacc.Bacc(None, target_bir_lowering=False, debug=False)

    # ---- DRAM I/O (per-core contents; all cores run the same program) ----
    qT_d = nc.dram_tensor("qT", [D, NQ], BF16, kind="ExternalInput")
    # packed bf16 consts: [0:128] W1aT | [128:256] W1bT | [256:288] pT |
    # [288:320] pTm2 | [320:384] W2T | [384:386] W3s
    blob_d = nc.dram_tensor("blob", [128, 386], BF16, kind="ExternalInput")
    # packed f32r 2-partition consts: [0:2048] qnA | [2048:2080] pn2
    qna_d = nc.dram_tensor("qna", [2, NQ + NPC], F32R, kind="ExternalInput")
    # packed f32 bias columns: b1c | b2s | b3c
    bcol_d = nc.dram_tensor("bcol", [128, 3], F32, kind="ExternalInput")
    out_d = nc.dram_tensor("out", [NQ, NPC], F32, kind="ExternalOutput")

    with tile.TileContext(nc) as tc, ExitStack() as ctx:
        consts = ctx.enter_context(tc.tile_pool(name="consts", bufs=1))
        h1pool = ctx.enter_context(tc.tile_pool(name="h1pool", bufs=6))
        h2pool = ctx.enter_context(tc.tile_pool(name="h2pool", bufs=NPAIR + 2))
        dpool = ctx.enter_context(tc.tile_pool(name="dpool", bufs=2))
        pmm = ctx.enter_context(tc.tile_pool(name="pmm", bufs=3, space="PSUM"))
        pres = ctx.enter_context(tc.tile_pool(name="pres", bufs=1, space="PSUM"))

        # ---- load constants (3 packed DMAs + 4 qT chunks) ----
        qT_sb = consts.tile([D, NQ], BF16)
        blob_sb = consts.tile([128, 386], BF16)
        qna_sb = consts.tile([2, NQ + NPC], F32R)
        bcol_sb = consts.tile([128, 3], F32)
        one_sb = consts.tile([128, 1], F32)
        e_sb = consts.tile([128, 1], F32)
        nc.sync.dma_start(out=blob_sb, in_=blob_d.ap())
        nc.scalar.dma_start(out=bcol_sb, in_=bcol_d.ap())
        for c in range(4):
            sl = slice(512 * c, 512 * (c + 1))
            eng = nc.sync if c % 2 == 0 else nc.scalar
            eng.dma_start(out=qT_sb[:, sl], in_=qT_d.ap()[:, sl])
        nc.scalar.dma_start(out=qna_sb, in_=qna_d.ap())
        nc.vector.memset(one_sb, 1.0)
        nc.vector.memset(e_sb, float(np.exp(1.0)))
        W1aT_sb = blob_sb[:, 0:128]
        W1bT_sb = blob_sb[:, 128:256]
        pT_sb = blob_sb[:, 256:288]
        pTm2_sb = blob_sb[:, 288:320]
        W2T_sb = blob_sb[:, 320:384]
        W3s_sb = blob_sb[:, 384:386]
        qnA_sb = qna_sb[:, 0:NQ]
        pn2_sb = qna_sb[:, NQ:NQ + NPC]
        b1c_sb = bcol_sb[:, 0:1]
        b2s_sb = bcol_sb[:, 1:2]
        b3c_sb = bcol_sb[:, 2:3]

        # ---- prolog: qhT (bf16, all queries), phb ----
        qhT_sb = consts.tile([H1, NQ], BF16)
        for c in range(4):
            qh_full = pmm.tile([128, 1024], F32, tag="ps")
            qh_ps = qh_full[:H1, :512]
            nc.tensor.matmul(qh_ps, W1aT_sb,
                             qT_sb[:, 512 * c: 512 * (c + 1)],
                             start=True, stop=True)
            nc.scalar.activation(out=qhT_sb[:, 512 * c: 512 * (c + 1)],
                                 in_=qh_ps, func=AF.Copy)

        ph_full = pmm.tile([128, 1024], F32, tag="ps")
        ph_ps = ph_full[:H1, :NPC]
        nc.tensor.matmul(ph_ps, W1bT_sb, pT_sb, start=True, stop=True)
        phb_sb = consts.tile([H1, NPC], F32)
        nc.vector.tensor_scalar(
            out=phb_sb, in0=ph_ps, scalar1=b1c_sb, scalar2=None, op0=OP.add)

        for _rep in range(nrep):
            # ---- base = qn + pn - 2 q.p ----
            # base_ps[:, 32*qc + pl] = base[q = 128*qc + part, p = pl]
            base_ps = pres.tile([128, 512], F32)
            for qc in range(NQCHUNK):
                dst = base_ps[:, NPC * qc: NPC * (qc + 1)]
                nc.tensor.matmul(dst, qT_sb[:, 128 * qc: 128 * (qc + 1)],
                                 pTm2_sb, start=True, stop=False)
                nc.tensor.matmul(dst, qnA_sb[:, 128 * qc: 128 * (qc + 1)],
                                 pn2_sb, start=False, stop=True)

            rb = dpool.tile([128, 512], F32, tag="rb")
            nc.vector.tensor_scalar(out=rb, in0=base_ps[:], scalar1=0.0,
                                    scalar2=None, op0=OP.max)

            ev_ps = pres.tile([128, 512], F32)

            # ---- main loop over local prototype pairs ----
            pending = []

            def flush_mm2():
                if SKIP_MM2:
                    pending.clear()
                    return
                for pair, h2t in pending:
                    for qc in range(NQCHUNK):
                        nc.tensor.matmul(
                            ev_ps[:, NPC * qc + 2 * pair: NPC * qc + 2 * pair + 2],
                            h2t[:, 128 * qc: 128 * (qc + 1)],
                            W3s_sb,
                            start=True, stop=True,
                        )
                pending.clear()

            for pair in range(NPAIR):
                pe, po = 2 * pair, 2 * pair + 1
                h1e = h1pool.tile([128, NQ], BF16, tag="h1e")
                h1o = h1pool.tile([128, NQ], BF16, tag="h1o")
                nc.vector.tensor_scalar(
                    out=h1e, in0=qhT_sb, scalar1=phb_sb[:, pe: pe + 1],
                    scalar2=0.0, op0=OP.add, op1=OP.max)
                nc.vector.tensor_scalar(
                    out=h1o, in0=qhT_sb, scalar1=phb_sb[:, po: po + 1],
                    scalar2=0.0, op0=OP.add, op1=OP.max)
                h2t = h2pool.tile([128, NQ], BF16, tag="h2")
                for half in range(2):           # 1024-q block -> 2 psum banks
                    ps = pmm.tile([128, 1024], F32, tag="ps")
                    sl = slice(1024 * half, 1024 * (half + 1))
                    nc.tensor.matmul(ps[0:64, 0:512], W2T_sb,
                                     h1e[:, 1024 * half: 1024 * half + 512],
                                     start=True, stop=True)
                    nc.tensor.matmul(ps[64:128, 0:512], W2T_sb,
                                     h1o[:, 1024 * half: 1024 * half + 512],
                                     start=True, stop=True, tile_position=(0, 64))
                    nc.tensor.matmul(ps[0:64, 512:1024], W2T_sb,
                                     h1e[:, 1024 * half + 512: 1024 * (half + 1)],
                                     start=True, stop=True)
                    nc.tensor.matmul(ps[64:128, 512:1024], W2T_sb,
                                     h1o[:, 1024 * half + 512: 1024 * (half + 1)],
                                     start=True, stop=True, tile_position=(0, 64))
                    if pair % 4 == 3:   # rebalance: every 4th pair's relu on DVE
                        nc.vector.tensor_scalar(
                            out=h2t[:, sl], in0=ps, scalar1=b2s_sb,
                            scalar2=0.0, op0=OP.add, op1=OP.max)
                    else:
                        nc.scalar.activation(out=h2t[:, sl], in_=ps, func=AF.Relu,
                                             bias=b2s_sb, scale=1.0)
                pending.append((pair, h2t))
                if pair % MM2_BATCH == MM2_BATCH - 1:
                    flush_mm2()
            flush_mm2()

            # ---- stage D: out = relu(base) * (softplus(ev + b3) + 1) ----
            # conf = ln(e*exp(ev+b3) + e) = softplus(ev+b3) + 1 (one ACT op)
            ngr = 2
            w = 512 // ngr
            for g in range(ngr):
                gs = slice(w * g, w * (g + 1))
                t = dpool.tile([128, w], F32, tag="t")
                nc.scalar.activation(out=t, in_=ev_ps[:, gs], func=AF.Exp,
                                     bias=b3c_sb, scale=1.0)
                u1 = dpool.tile([128, w], F32, tag="u1")
                nc.scalar.activation(out=u1, in_=t, func=AF.Ln,
                                     bias=e_sb[:, 0:1],
                                     scale=e_sb[:, 0:1])
                ot = dpool.tile([128, w], F32, tag="ot")
                nc.vector.tensor_mul(ot, rb[:, gs], u1)
                nqc_g = w // NPC
                out_v = out_d.ap()[128 * nqc_g * g: 128 * nqc_g * (g + 1), :] \
                    .rearrange("(c q) p -> q c p", c=nqc_g)
                eng = nc.sync if g % 2 == 0 else nc.scalar
                eng.dma_start(out=out_v,
                              in_=ot.rearrange("q (c p) -> q c p", c=nqc_g))

    nc.compile()
    return nc


def make_in_maps(query_features, prototypes, W1, b1, W2, b2, W3, b3):
    q = np.asarray(query_features, dtype=np.float32)
    p = np.asarray(prototypes, dtype=np.float32)
    W1 = np.asarray(W1, dtype=np.float32)
    W2 = np.asarray(W2, dtype=np.float32)
    W3 = np.asarray(W3, dtype=np.float32)
    b1 = np.asarray(b1, dtype=np.float32)
    b2 = np.asarray(b2, dtype=np.float32)
    b3 = np.asarray(b3, dtype=np.float32)

    qT = np.ascontiguousarray(q.T)                        # [D, NQ]
    qn = (q * q).sum(1)
    pn = (p * p).sum(1)
    w3s = np.zeros((128, 2), np.float32)
    w3s[0:64, 0] = W3[0]
    w3s[64:128, 1] = W3[0]
    bcol = np.stack([b1, np.concatenate([b2, b2]),
                     np.full(128, b3[0], np.float32)], axis=1).astype(np.float32)
    common = {
        "qT": qT.astype(ml_dtypes.bfloat16),
        "bcol": np.ascontiguousarray(bcol),
    }
    blob = np.zeros((128, 386), np.float32)
    blob[:, 0:128] = W1[:, :D].T
    blob[:, 128:256] = W1[:, D:].T
    blob[:, 320:384] = W2.T
    blob[:, 384:386] = w3s

    in_maps = []
    for c in range(NCORES):
        sl = slice(c * NPC, (c + 1) * NPC)
        pTs = p[sl].T                                     # [D, NPC]
        bl = blob.copy()
        bl[:, 256:288] = pTs
        bl[:, 288:320] = -2.0 * pTs
        qna = np.concatenate(
            [np.stack([qn, np.ones(NQ, np.float32)]),
             np.stack([np.ones(NPC, np.float32), pn[sl]])], axis=1)
        m = dict(common)
        m["blob"] = bl.astype(ml_dtypes.bfloat16)
        m["qna"] = np.ascontiguousarray(qna.astype(np.float32))
        in_maps.append(m)
    return in_maps


class Runner:
    """Compile the bass program into a reusable 8-core jitted callable."""

    def __init__(self, nc):
        import jax
        import concourse.mybir as _mybir
        from concourse import bass2jax
        from jax.sharding import Mesh, PartitionSpec
        from jax.experimental.shard_map import shard_map

        bass2jax.install_neuronx_cc_hook()
        self.nc = nc
        partition_name = nc.partition_id_tensor.name if nc.partition_id_tensor else None
        in_names, out_names, out_avals = [], [], []
        for alloc in nc.m.functions[0].allocations:
            if not isinstance(alloc, _mybir.MemoryLocationSet):
                continue
            name = alloc.memorylocations[0].name
            if alloc.kind == "ExternalInput":
                if name != partition_name:
                    in_names.append(name)
            elif alloc.kind == "ExternalOutput":
                out_names.append(name)
                out_avals.append(jax.core.ShapedArray(
                    tuple(alloc.tensor_shape), _mybir.dt.np(alloc.dtype)))
        self.in_names, self.out_names, self.out_avals = in_names, out_names, out_avals
        n_params, n_outs = len(in_names), len(out_names)
        all_names = in_names + out_names
        if partition_name is not None:
            all_names = all_names + [partition_name]

        def _body(*args):
            operands = list(args)
            if partition_name is not None:
                operands.append(bass2jax.partition_id_tensor())
            outs = bass2jax._bass_exec_p.bind(
                *operands,
                out_avals=tuple(out_avals),
                in_names=tuple(all_names),
                out_names=tuple(out_names),
                lowering_input_output_aliases=(),
                sim_require_finite=True,
                sim_require_nnan=True,
                nc=nc,
            )
            return tuple(outs)

        devices = jax.devices()[:NCORES]
        mesh = Mesh(np.asarray(devices), ("core",))
        self.jit = jax.jit(
            shard_map(_body, mesh=mesh,
                      in_specs=(PartitionSpec("core"),) * (n_params + n_outs),
                      out_specs=(PartitionSpec("core"),) * n_outs,
                      check_rep=False),
            keep_unused=True,
        )
        self._jax = jax

    def prep(self, in_maps):
        concat_in = [
            np.concatenate([m[name] for m in in_maps], axis=0)
            for name in self.in_names
        ]
        concat_zeros = [
            np.zeros((NCORES * a.shape[0], *a.shape[1:]), a.dtype)
            for a in self.out_avals
        ]
        return [self._jax.device_put(x) for x in concat_in + concat_zeros]

    def exec(self, args):
        return self._jax.block_until_ready(self.jit(*args))

    def run(self, in_maps):
        outs = self.exec(self.prep(in_maps))
        return [
            {name: np.asarray(outs[i]).reshape(NCORES, *self.out_avals[i].shape)[c]
             for i, name in enumerate(self.out_names)}
            for c in range(NCORES)
        ]


def get_runner(nrep=1):
    key = f"runner{nrep}"
    if key not in _CACHE:
        _CACHE[key] = Runner(build_bass(nrep))
    return _CACHE[key]


def run(inputs, trace=False, **kw):
    runner = get_runner()
    in_maps = make_in_maps(**inputs)
    results = runner.run(in_maps)
    out = np.concatenate([results[c]["out"] for c in range(NCORES)], axis=1)
    return out.astype(np.float32), results


def kernel(**inputs) -> np.ndarray:
    out, _ = run(inputs)
    return out


if __name__ == "__main__":
    rng = np.random.default_rng(0)
    s1, s2, s3 = 1 / np.sqrt(2 * D), 1 / np.sqrt(H1), 1 / np.sqrt(H2)
    ins = {
        "query_features": rng.standard_normal((NQ, D)).astype(np.float32),
        "prototypes": rng.standard_normal((NP, D)).astype(np.float32),
        "W1": rng.uniform(-s1, s1, (H1, 2 * D)).astype(np.float32),
        "b1": rng.uniform(-s1, s1, (H1,)).astype(np.float32),
        "W2": rng.uniform(-s2, s2, (H2, H1)).astype(np.float32),
        "b2": rng.uniform(-s2, s2, (H2,)).astype(np.float32),
        "W3": rng.uniform(-s3, s3, (1, H2)).astype(np.float32),
        "b3": rng.uniform(-s3, s3, (1,)).astype(np.float32),
    }
    out = kernel(**ins)
    q, p = ins["query_features"], ins["prototypes"]
    qh = q @ ins["W1"][:, :D].T
    ph = p @ ins["W1"][:, D:].T
    h1 = np.maximum(qh[:, None, :] + ph[None, :, :] + ins["b1"], 0)
    h2 = np.maximum(h1.reshape(-1, H1) @ ins["W2"].T + ins["b2"], 0)
    z = (h2 @ ins["W3"][0]).reshape(NQ, NP) + ins["b3"][0]
    evd = np.log1p(np.exp(z))
    basem = np.maximum((q * q).sum(1)[:, None] + (p * p).sum(1)[None, :]
                       - 2 * q @ p.T, 0)
    ref = basem / (1.0 / (evd + 1.0) + 1e-8)
    err = np.linalg.norm(out - ref) / np.linalg.norm(ref)
    maxrel = np.max(np.abs(out - ref) / (np.abs(ref) + 1e-6))
    print("norm rel err:", err, " max rel err:", maxrel)
